# revision 17
# baseline (speedup 1.0000x reference)
"""Trainium2 Bass kernel for the KnowledgeGraphEmbedding loss.

Computes, for P=1024 relations sharded 128-per-core across 8 NeuronCores:
    li = Lp_w[p] @ wi          (wi = tag_rep[tag1_idx])
    rj = Rp_w[p] @ wj          (wj = tag_rep[tag2_idx])
    dist[p] = sum_h (li - rj)^2
    out = [dist*rel, dist*(1-rel), rel, 1-rel]   (rel in {0,1})

Memory-bound problem: all of Lp_w/Rp_w must stream from HBM once.
Strategy vs the f32 DVE baseline (~305us, at the f32 DMA roofline):

  1. Quantize weights host-side to fp8 e4m3 scaled by 256 into range;
     wi/-wj scaled by 32. Quarters HBM traffic to ~23.7 MB/core, which
     streams gap-free at 360-420 GB/s (the HBM/fabric ceiling; exact
     rate depends on phase alignment with the stack-paired core).
     End-to-end max-abs rel err 1.025e-2, deterministic (gate 2e-2).
  2. Multiply+reduce on the TensorEngine as a segmented matvec in
     DoubleRow mode (2 fp8 weights per PE cell = virtual 256-row array):
     stationary lhsT[128, 2, 5] is a block-diagonal of a 50-element
     wi/-wj e-chunk (5 groups of 2x25), moving rhs[128, 2, 512] carries
     the matching e-fragments of 5 h-rows x 512 (hg, rel) columns,
     host-pretransposed (rows 125-127 zero so the 128-partition DMA
     spray engages all 16 SDMA engines). 12 matmuls (2 sides x 6
     e-chunks) accumulate diff = li - rj in one PSUM bank per batch.
  3. ScalarE squares PSUM (scale 1/8192 folds the quant scales), DVE
     tensor_reduce folds the hg axis, one final f32 matmul against ones
     folds the 5-partition axis to dist[128, 1]; mask/bin via DVE.
  Dummy matmuls at the start keep the PE HAM clock-gate warm while the
  first batches stream in.
"""

from contextlib import ExitStack

import numpy as np

N_CORES = 8
P_TOTAL = 1024
H = 300
E = 300
P_LOC = P_TOTAL // N_CORES  # 128 relations per core

G = 25        # e-fragment length per block-diag group per plane
M = 5         # h-rows packed per column set (stationary free dim)
J = 2         # DoubleRow planes
KU = G * M    # 125 useful contraction rows per plane
K = 128       # padded contraction rows (zero rows 125-127)
NCH = E // (G * J)  # 6 e-chunks of 50
HG = H // M   # 60 h-groups
HB = 4        # h-groups per batch
NB = HG // HB  # 15 batches
NCOL = HB * P_LOC  # 512 output columns per matmul (= one PSUM bank f32)
STP = 16      # stationary plane stride (DoubleRow wants step % 16 == 0)

ALPHA = 256.0  # weight scale into e4m3 range
BETA = 32.0    # wi/wj scale into e4m3 range
INV_SCALE = 1.0 / (ALPHA * BETA)
F8_MAX = 240.0

# Set by test harness to capture a profile; kernel() stores results here.
TRACE = False
LAST_RESULT = None

_CACHE: dict = {}


def _build_nc():
    import concourse.bacc as bacc
    import concourse.mybir as mybir
    import concourse.tile as tile

    f32 = mybir.dt.float32
    f8 = mybir.dt.float8e4

    nc = bacc.Bacc("TRN2", debug=False)

    # Moving operands, host-pretransposed/quantized:
    #   row b*K+k, col c*(J*NCOL) + j*NCOL + hg_l*P_LOC + rel
    #     = Wq[rel, (b*HB+hg_l)*M + k//G, c*G*J + j*G + k%G]  (k < 125)
    lx = nc.dram_tensor("lx", [NB * K, NCH * J * NCOL], f8, kind="ExternalInput").ap()
    rx = nc.dram_tensor("rx", [NB * K, NCH * J * NCOL], f8, kind="ExternalInput").ap()
    # Block-diagonal stationaries: [128, (side*NCH+c)*2*STP + j*STP + m]
    st = nc.dram_tensor("st", [K, 2 * NCH * J * STP], f8, kind="ExternalInput").ap()
    rm = nc.dram_tensor("rm", [P_LOC, 2], f32, kind="ExternalInput").ap()
    out = nc.dram_tensor("out", [P_LOC, 4], f32, kind="ExternalOutput").ap()

    with tile.TileContext(nc) as tc, ExitStack() as ctx:
        const_pool = ctx.enter_context(tc.tile_pool(name="const", bufs=1))
        lpool = ctx.enter_context(tc.tile_pool(name="lmov", bufs=8))
        rpool = ctx.enter_context(tc.tile_pool(name="rmov", bufs=8))
        spool = ctx.enter_context(tc.tile_pool(name="sq", bufs=3))
        fpool = ctx.enter_context(tc.tile_pool(name="fold", bufs=3))
        psum_pool = ctx.enter_context(tc.tile_pool(name="ps", bufs=4, space="PSUM"))
        warm_pool = ctx.enter_context(tc.tile_pool(name="wps", bufs=1, space="PSUM"))
        dist_pool = ctx.enter_context(tc.tile_pool(name="dps", bufs=1, space="PSUM"))

        st_sb = const_pool.tile([K, 2 * NCH * J * STP], f8)
        nc.scalar.dma_start(st_sb[:], st[:])
        rm_sb = const_pool.tile([P_LOC, 2], f32)
        nc.scalar.dma_start(rm_sb[:], rm[:])
        ones = const_pool.tile([M, 1], f32)
        nc.vector.memset(ones[:], 1.0)
        sacc = const_pool.tile([M, P_LOC], f32)
        nc.vector.memset(sacc[:], 0.0)

        # HAM pre-warm: dummy matmuls bridge the gap until the first batch
        # lands, so the PE clock-gate reaches 8/8 as real work starts. Kept
        # short: a long warm block would stall the PE FIFO while DMA fills
        # every pool slot and then idles on slot recycling.
        warm_ps = warm_pool.tile([M, 128], f32)
        for _ in range(14):
            nc.tensor.matmul(
                warm_ps[:], st_sb[:, 0:M], st_sb[:, 0:128], start=True, stop=True
            )

        dist_ps = dist_pool.tile([P_LOC, 1], f32)
        CW = J * NCOL  # 1024 columns per chunk
        for b in range(NB - 1):
            xt_l = lpool.tile([K, NCH * CW], f8, name="xt_l", tag="xt_l")
            # Both sides on the SP HWDGE ring: the ACT ring's issues would
            # queue behind ACT's evac semaphore waits. (Splitting the last
            # batch into piece-tiles was tried and measured slower: per-piece
            # issue overhead and small-transfer inefficiency at the stream
            # tail outweigh the finer matmul dependencies.)
            nc.sync.dma_start(xt_l[:], lx[b * K : (b + 1) * K, :])
            xt_r = rpool.tile([K, NCH * CW], f8, name="xt_r", tag="xt_r")
            nc.sync.dma_start(xt_r[:], rx[b * K : (b + 1) * K, :])
            r_src = lambda c: xt_r[:, c * CW : (c + 1) * CW]

            ps = psum_pool.tile([M, NCOL], f32, name="ps", tag="ps")
            n = 0
            for s in (0, 1):
                for c in range(NCH):
                    base = (s * NCH + c) * J * STP
                    lhsT = st_sb[:, base : base + J * STP].rearrange(
                        "p (j x) -> p j x", j=J
                    )[:, :, 0:M]
                    mov = xt_l[:, c * CW : (c + 1) * CW] if s == 0 else r_src(c)
                    nc.tensor.matmul(
                        ps[:],
                        lhsT,
                        mov.rearrange("p (j n) -> p j n", j=J),
                        start=(n == 0),
                        stop=(n == 2 * NCH - 1),
                        perf_mode=mybir.MatmulPerfMode.DoubleRow,
                    )
                    n += 1

            sq = spool.tile([M, NCOL], f32)
            nc.scalar.activation(
                sq[:], ps[:], mybir.ActivationFunctionType.Square, scale=INV_SCALE
            )
            # sq layout [m, (hg_l, rel)]; reduce hg_l (stride P_LOC) per rel.
            fold = fpool.tile([M, P_LOC], f32)
            sq_v = sq.rearrange("m (g r) -> m r g", g=HB)
            nc.vector.tensor_reduce(
                fold[:], sq_v, mybir.AxisListType.X, mybir.AluOpType.add
            )
            nc.vector.tensor_add(sacc[:], sacc[:], fold[:])

        # Final batch as two half-batches (2 h-groups, N=256): halves the
        # serial matmul+evac+reduce chain left after the stream's last byte.
        # Host packs the last row-block as [halfA | halfB] columns.
        HW2 = NCH * J * 256  # 3072 columns per half
        for hh in range(2):
            xt_lh = lpool.tile([K, HW2], f8, name="xt_lh", tag="xt_l")
            xt_rh = rpool.tile([K, HW2], f8, name="xt_rh", tag="xt_r")
            b0 = (NB - 1) * K
            nc.sync.dma_start(xt_lh[:], lx[b0 : b0 + K, hh * HW2 : (hh + 1) * HW2])
            nc.sync.dma_start(xt_rh[:], rx[b0 : b0 + K, hh * HW2 : (hh + 1) * HW2])
            psh = psum_pool.tile([M, 256], f32, name="psh", tag="ps")
            n = 0
            for s in (0, 1):
                xt = xt_lh if s == 0 else xt_rh
                for c in range(NCH):
                    base = (s * NCH + c) * J * STP
                    lhsT = st_sb[:, base : base + J * STP].rearrange(
                        "p (j x) -> p j x", j=J
                    )[:, :, 0:M]
                    rhs = xt[:, c * 512 : (c + 1) * 512].rearrange(
                        "p (j n) -> p j n", j=J
                    )
                    nc.tensor.matmul(
                        psh[:],
                        lhsT,
                        rhs,
                        start=(n == 0),
                        stop=(n == 2 * NCH - 1),
                        perf_mode=mybir.MatmulPerfMode.DoubleRow,
                    )
                    n += 1
            sqh = spool.tile([M, 256], f32, name="sqh", tag="sq")
            nc.scalar.activation(
                sqh[:], psh[:], mybir.ActivationFunctionType.Square, scale=INV_SCALE
            )
            foldh = fpool.tile([M, P_LOC], f32, name="foldh", tag="fold")
            nc.vector.tensor_reduce(
                foldh[:],
                sqh.rearrange("m (g r) -> m r g", g=2),
                mybir.AxisListType.X,
                mybir.AluOpType.add,
            )
            if hh == 0:
                nc.vector.tensor_add(sacc[:], sacc[:], foldh[:])
                # Bulk of dist folds while the final half-batch drains.
                nc.tensor.matmul(dist_ps[:], sacc[:], ones[:], start=True, stop=False)
            else:
                nc.tensor.matmul(dist_ps[:], foldh[:], ones[:], start=False, stop=True)

        dist_sb = const_pool.tile([P_LOC, 1], f32)
        nc.vector.tensor_copy(dist_sb[:], dist_ps[:])

        out_sb = const_pool.tile([P_LOC, 4], f32)
        nc.vector.tensor_scalar_mul(out_sb[:, 0:2], rm_sb[:, 0:2], dist_sb[:, 0:1])
        nc.vector.tensor_copy(out_sb[:, 2:4], rm_sb[:, 0:2])
        nc.sync.dma_start(out[:], out_sb[:])

    nc.compile()
    return nc


def _quant_f8(x):
    import ml_dtypes

    return np.clip(x, -F8_MAX, F8_MAX).astype(ml_dtypes.float8_e4m3)


def _prepack_side(wq_core):
    """[128, 300, 300] e4m3 -> [NB*K, NCH*J*NCOL] moving layout, K-padded."""
    # e = c*(G*J) + j*G + i
    a = wq_core.reshape(P_LOC, NB, HB, M, NCH, J, G)  # rel, b, hg_l, m, c, j, i
    a = a.transpose(1, 3, 6, 4, 5, 2, 0)  # b, m, i, c, j, hg_l, rel
    x = np.zeros((NB, K, NCH * J * NCOL), dtype=wq_core.dtype)
    x[:, :KU] = np.ascontiguousarray(a).reshape(NB, KU, NCH * J * NCOL)
    # Last block: repack columns (c, j, hg_l, rel) -> [halfA | halfB] with
    # hg_l 0-1 in halfA, 2-3 in halfB (kernel runs it as two half-batches).
    blk = x[NB - 1].reshape(K, NCH, J, HB, P_LOC)
    ha = np.ascontiguousarray(blk[:, :, :, 0:2, :]).reshape(K, NCH * J * 2 * P_LOC)
    hb = np.ascontiguousarray(blk[:, :, :, 2:4, :]).reshape(K, NCH * J * 2 * P_LOC)
    x[NB - 1] = np.concatenate([ha, hb], axis=1)
    return x.reshape(NB * K, NCH * J * NCOL)


def kernel(tag_rep, Lp_w, Rp_w, relation, tag1_idx, tag2_idx):
    global LAST_RESULT
    from concourse.bass_utils import run_bass_kernel_spmd

    if "nc" not in _CACHE:
        _CACHE["nc"] = _build_nc()
    nc = _CACHE["nc"]

    tag_rep = np.asarray(tag_rep)
    rel = np.asarray(relation).astype(np.float32)  # values in {0, 1}

    wi = np.asarray(tag_rep[int(tag1_idx)], dtype=np.float32)
    wj = np.asarray(tag_rep[int(tag2_idx)], dtype=np.float32)
    v_l = _quant_f8(wi * BETA)
    v_r = _quant_f8(-wj * BETA)

    st = np.zeros((K, 2 * NCH * J * STP), dtype=v_l.dtype)
    for s, v in ((0, v_l), (1, v_r)):
        for c in range(NCH):
            for j in range(J):
                for m in range(M):
                    e0 = c * G * J + j * G
                    st[m * G : (m + 1) * G, (s * NCH + c) * J * STP + j * STP + m] = v[
                        e0 : e0 + G
                    ]

    lq = _quant_f8(np.asarray(Lp_w) * ALPHA)
    rq = _quant_f8(np.asarray(Rp_w) * ALPHA)

    in_maps = []
    for core in range(N_CORES):
        sl = slice(core * P_LOC, (core + 1) * P_LOC)
        rel_c = rel[sl]
        in_maps.append(
            {
                "lx": _prepack_side(lq[sl]),
                "rx": _prepack_side(rq[sl]),
                "st": st,
                "rm": np.ascontiguousarray(np.stack([rel_c, 1.0 - rel_c], axis=1)),
            }
        )

    kw = {}
    if TRACE:
        kw = dict(trace=True, trace_cores=[0])
    res = run_bass_kernel_spmd(nc, in_maps, core_ids=list(range(N_CORES)), **kw)
    LAST_RESULT = res

    out_full = np.empty((4, P_TOTAL), dtype=np.float32)
    for core in range(N_CORES):
        out_full[:, core * P_LOC : (core + 1) * P_LOC] = res.results[core]["out"].T
    return out_full


# revision 19
# speedup vs baseline: 1.0017x; 1.0017x over previous
"""Trainium2 Bass kernel for the KnowledgeGraphEmbedding loss.

Computes, for P=1024 relations sharded 128-per-core across 8 NeuronCores:
    li = Lp_w[p] @ wi          (wi = tag_rep[tag1_idx])
    rj = Rp_w[p] @ wj          (wj = tag_rep[tag2_idx])
    dist[p] = sum_h (li - rj)^2
    out = [dist*rel, dist*(1-rel), rel, 1-rel]   (rel in {0,1})

Memory-bound problem: all of Lp_w/Rp_w must stream from HBM once.
Strategy vs the f32 DVE baseline (~305us, at the f32 DMA roofline):

  1. Quantize weights host-side to fp8 e4m3 scaled by 256 into range;
     wi/-wj scaled by 32. Quarters HBM traffic to ~23.7 MB/core, which
     streams gap-free at 360-420 GB/s (the HBM/fabric ceiling; exact
     rate depends on phase alignment with the stack-paired core).
     End-to-end max-abs rel err 1.025e-2, deterministic (gate 2e-2).
  2. Multiply+reduce on the TensorEngine as a segmented matvec in
     DoubleRow mode (2 fp8 weights per PE cell = virtual 256-row array):
     stationary lhsT[128, 2, 5] is a block-diagonal of a 50-element
     wi/-wj e-chunk (5 groups of 2x25), moving rhs[128, 2, 512] carries
     the matching e-fragments of 5 h-rows x 512 (hg, rel) columns,
     host-pretransposed (rows 125-127 zero so the 128-partition DMA
     spray engages all 16 SDMA engines). 12 matmuls (2 sides x 6
     e-chunks) accumulate diff = li - rj in one PSUM bank per batch.
  3. ScalarE squares PSUM (scale 1/8192 folds the quant scales), DVE
     tensor_reduce folds the hg axis, one final f32 matmul against ones
     folds the 5-partition axis to dist[128, 1]; mask/bin via DVE.
  Dummy matmuls at the start keep the PE HAM clock-gate warm while the
  first batches stream in.
"""

from contextlib import ExitStack

import numpy as np

N_CORES = 8
P_TOTAL = 1024
H = 300
E = 300
P_LOC = P_TOTAL // N_CORES  # 128 relations per core

G = 25        # e-fragment length per block-diag group per plane
M = 5         # h-rows packed per column set (stationary free dim)
J = 2         # DoubleRow planes
KU = G * M    # 125 useful contraction rows per plane
K = 128       # padded contraction rows (zero rows 125-127)
NCH = E // (G * J)  # 6 e-chunks of 50
HG = H // M   # 60 h-groups
HB = 4        # h-groups per batch
NB = HG // HB  # 15 batches
NCOL = HB * P_LOC  # 512 output columns per matmul (= one PSUM bank f32)
STP = 16      # stationary plane stride (DoubleRow wants step % 16 == 0)

ALPHA = 256.0  # weight scale into e4m3 range
BETA = 32.0    # wi/wj scale into e4m3 range
INV_SCALE = 1.0 / (ALPHA * BETA)
F8_MAX = 240.0

# Set by test harness to capture a profile; kernel() stores results here.
TRACE = False
LAST_RESULT = None

_CACHE: dict = {}


def _build_nc():
    import concourse.bacc as bacc
    import concourse.mybir as mybir
    import concourse.tile as tile

    f32 = mybir.dt.float32
    f8 = mybir.dt.float8e4

    nc = bacc.Bacc("TRN2", debug=False)

    # Moving operands, host-pretransposed/quantized:
    #   row b*K+k, col c*(J*NCOL) + j*NCOL + hg_l*P_LOC + rel
    #     = Wq[rel, (b*HB+hg_l)*M + k//G, c*G*J + j*G + k%G]  (k < 125)
    lx = nc.dram_tensor("lx", [NB * K, NCH * J * NCOL], f8, kind="ExternalInput").ap()
    rx = nc.dram_tensor("rx", [NB * K, NCH * J * NCOL], f8, kind="ExternalInput").ap()
    # Block-diagonal stationaries: [128, (side*NCH+c)*2*STP + j*STP + m]
    st = nc.dram_tensor("st", [K, 2 * NCH * J * STP], f8, kind="ExternalInput").ap()
    rm = nc.dram_tensor("rm", [P_LOC, 2], f32, kind="ExternalInput").ap()
    out = nc.dram_tensor("out", [P_LOC, 4], f32, kind="ExternalOutput").ap()

    with tile.TileContext(nc) as tc, ExitStack() as ctx:
        const_pool = ctx.enter_context(tc.tile_pool(name="const", bufs=1))
        lpool = ctx.enter_context(tc.tile_pool(name="lmov", bufs=8))
        rpool = ctx.enter_context(tc.tile_pool(name="rmov", bufs=8))
        spool = ctx.enter_context(tc.tile_pool(name="sq", bufs=3))
        fpool = ctx.enter_context(tc.tile_pool(name="fold", bufs=3))
        psum_pool = ctx.enter_context(tc.tile_pool(name="ps", bufs=4, space="PSUM"))
        warm_pool = ctx.enter_context(tc.tile_pool(name="wps", bufs=1, space="PSUM"))
        dist_pool = ctx.enter_context(tc.tile_pool(name="dps", bufs=1, space="PSUM"))

        st_sb = const_pool.tile([K, 2 * NCH * J * STP], f8)
        nc.scalar.dma_start(st_sb[:], st[:])
        rm_sb = const_pool.tile([P_LOC, 2], f32)
        nc.scalar.dma_start(rm_sb[:], rm[:])
        ones = const_pool.tile([M, 1], f32)
        nc.vector.memset(ones[:], 1.0)
        sacc = const_pool.tile([M, P_LOC], f32)
        nc.vector.memset(sacc[:], 0.0)

        # HAM pre-warm: dummy matmuls bridge the gap until the first batch
        # lands, so the PE clock-gate reaches 8/8 as real work starts. Kept
        # short: a long warm block would stall the PE FIFO while DMA fills
        # every pool slot and then idles on slot recycling.
        warm_ps = warm_pool.tile([M, 128], f32)
        for _ in range(14):
            nc.tensor.matmul(
                warm_ps[:], st_sb[:, 0:M], st_sb[:, 0:128], start=True, stop=True
            )

        dist_ps = dist_pool.tile([P_LOC, 1], f32)
        CW = J * NCOL  # 1024 columns per chunk
        for b in range(NB):
            last = b == NB - 1
            xt_l = lpool.tile([K, NCH * CW], f8, name="xt_l", tag="xt_l")
            # Both sides on the SP HWDGE ring: the ACT ring's issues would
            # queue behind ACT's evac semaphore waits. (Splitting the last
            # batch into piece-tiles was tried and measured slower: per-piece
            # issue overhead and small-transfer inefficiency at the stream
            # tail outweigh the finer matmul dependencies.)
            nc.sync.dma_start(xt_l[:], lx[b * K : (b + 1) * K, :])
            xt_r = rpool.tile([K, NCH * CW], f8, name="xt_r", tag="xt_r")
            nc.sync.dma_start(xt_r[:], rx[b * K : (b + 1) * K, :])
            r_src = lambda c: xt_r[:, c * CW : (c + 1) * CW]

            ps = psum_pool.tile([M, NCOL], f32, name="ps", tag="ps")
            n = 0
            for s in (0, 1):
                for c in range(NCH):
                    base = (s * NCH + c) * J * STP
                    lhsT = st_sb[:, base : base + J * STP].rearrange(
                        "p (j x) -> p j x", j=J
                    )[:, :, 0:M]
                    mov = xt_l[:, c * CW : (c + 1) * CW] if s == 0 else r_src(c)
                    nc.tensor.matmul(
                        ps[:],
                        lhsT,
                        mov.rearrange("p (j n) -> p j n", j=J),
                        start=(n == 0),
                        stop=(n == 2 * NCH - 1),
                        perf_mode=mybir.MatmulPerfMode.DoubleRow,
                    )
                    n += 1

            sq = spool.tile([M, NCOL], f32)
            nc.scalar.activation(
                sq[:], ps[:], mybir.ActivationFunctionType.Square, scale=INV_SCALE
            )
            # sq layout [m, (hg_l, rel)]; reduce hg_l (stride P_LOC) per rel.
            fold = fpool.tile([M, P_LOC], f32)
            sq_v = sq.rearrange("m (g r) -> m r g", g=HB)
            nc.vector.tensor_reduce(
                fold[:], sq_v, mybir.AxisListType.X, mybir.AluOpType.add
            )
            if last:
                # Fold the bulk of dist while the final batch drains: dist_ps
                # accumulates sacc (batches 0-13) early, then just this
                # batch's fold, skipping the last sacc add.
                nc.tensor.matmul(dist_ps[:], sacc[:], ones[:], start=True, stop=False)
                nc.tensor.matmul(dist_ps[:], fold[:], ones[:], start=False, stop=True)
            else:
                nc.vector.tensor_add(sacc[:], sacc[:], fold[:])

        out_sb = const_pool.tile([P_LOC, 4], f32)
        # Scalar operand read straight from PSUM: skips a dist copy to SBUF.
        nc.vector.tensor_scalar_mul(out_sb[:, 0:2], rm_sb[:, 0:2], dist_ps[:, 0:1])
        nc.vector.tensor_copy(out_sb[:, 2:4], rm_sb[:, 0:2])
        nc.sync.dma_start(out[:], out_sb[:])

    nc.compile()
    return nc


def _quant_f8(x):
    import ml_dtypes

    return np.clip(x, -F8_MAX, F8_MAX).astype(ml_dtypes.float8_e4m3)


def _prepack_side(wq_core):
    """[128, 300, 300] e4m3 -> [NB*K, NCH*J*NCOL] moving layout, K-padded."""
    # e = c*(G*J) + j*G + i
    a = wq_core.reshape(P_LOC, NB, HB, M, NCH, J, G)  # rel, b, hg_l, m, c, j, i
    a = a.transpose(1, 3, 6, 4, 5, 2, 0)  # b, m, i, c, j, hg_l, rel
    x = np.zeros((NB, K, NCH * J * NCOL), dtype=wq_core.dtype)
    x[:, :KU] = np.ascontiguousarray(a).reshape(NB, KU, NCH * J * NCOL)
    return x.reshape(NB * K, NCH * J * NCOL)


def kernel(tag_rep, Lp_w, Rp_w, relation, tag1_idx, tag2_idx):
    global LAST_RESULT
    from concourse.bass_utils import run_bass_kernel_spmd

    if "nc" not in _CACHE:
        _CACHE["nc"] = _build_nc()
    nc = _CACHE["nc"]

    tag_rep = np.asarray(tag_rep)
    rel = np.asarray(relation).astype(np.float32)  # values in {0, 1}

    wi = np.asarray(tag_rep[int(tag1_idx)], dtype=np.float32)
    wj = np.asarray(tag_rep[int(tag2_idx)], dtype=np.float32)
    v_l = _quant_f8(wi * BETA)
    v_r = _quant_f8(-wj * BETA)

    st = np.zeros((K, 2 * NCH * J * STP), dtype=v_l.dtype)
    for s, v in ((0, v_l), (1, v_r)):
        for c in range(NCH):
            for j in range(J):
                for m in range(M):
                    e0 = c * G * J + j * G
                    st[m * G : (m + 1) * G, (s * NCH + c) * J * STP + j * STP + m] = v[
                        e0 : e0 + G
                    ]

    lq = _quant_f8(np.asarray(Lp_w) * ALPHA)
    rq = _quant_f8(np.asarray(Rp_w) * ALPHA)

    in_maps = []
    for core in range(N_CORES):
        sl = slice(core * P_LOC, (core + 1) * P_LOC)
        rel_c = rel[sl]
        in_maps.append(
            {
                "lx": _prepack_side(lq[sl]),
                "rx": _prepack_side(rq[sl]),
                "st": st,
                "rm": np.ascontiguousarray(np.stack([rel_c, 1.0 - rel_c], axis=1)),
            }
        )

    kw = {}
    if TRACE:
        kw = dict(trace=True, trace_cores=[0])
    res = run_bass_kernel_spmd(nc, in_maps, core_ids=list(range(N_CORES)), **kw)
    LAST_RESULT = res

    out_full = np.empty((4, P_TOTAL), dtype=np.float32)
    for core in range(N_CORES):
        out_full[:, core * P_LOC : (core + 1) * P_LOC] = res.results[core]["out"].T
    return out_full


# revision 20
# speedup vs baseline: 1.1139x; 1.1119x over previous
"""Trainium2 Bass kernel for the KnowledgeGraphEmbedding loss.

Computes, for P=1024 relations sharded 128-per-core across 8 NeuronCores:
    li = Lp_w[p] @ wi          (wi = tag_rep[tag1_idx])
    rj = Rp_w[p] @ wj          (wj = tag_rep[tag2_idx])
    dist[p] = sum_h (li - rj)^2
    out = [dist*rel, dist*(1-rel), rel, 1-rel]   (rel in {0,1})

Memory-bound problem: all of Lp_w/Rp_w must stream from HBM once.
Strategy vs the f32 DVE baseline (~305us, at the f32 DMA roofline):

  1. Quantize weights host-side to fp8 e4m3 scaled by 256 into range;
     wi/-wj scaled by 32. Quarters HBM traffic to ~23.7 MB/core, which
     streams gap-free at 360-420 GB/s (the HBM/fabric ceiling; exact
     rate depends on phase alignment with the stack-paired core).
     End-to-end max-abs rel err 1.025e-2, deterministic (gate 2e-2).
  2. Multiply+reduce on the TensorEngine as a segmented matvec in
     DoubleRow mode (2 fp8 weights per PE cell = virtual 256-row array):
     stationary lhsT[128, 2, 5] is a block-diagonal of a 50-element
     wi/-wj e-chunk (5 groups of 2x25), moving rhs[128, 2, 512] carries
     the matching e-fragments of 5 h-rows x 512 (hg, rel) columns,
     host-pretransposed (rows 125-127 zero so the 128-partition DMA
     spray engages all 16 SDMA engines). 12 matmuls (2 sides x 6
     e-chunks) accumulate diff = li - rj in one PSUM bank per batch.
  3. ScalarE squares PSUM (scale 1/8192 folds the quant scales), DVE
     tensor_reduce folds the hg axis, one final f32 matmul against ones
     folds the 5-partition axis to dist[128, 1]; mask/bin via DVE.
  Dummy matmuls at the start keep the PE HAM clock-gate warm while the
  first batches stream in.
"""

from contextlib import ExitStack

import numpy as np

N_CORES = 8
P_TOTAL = 1024
H = 300
E = 300
P_LOC = P_TOTAL // N_CORES  # 128 relations per core

G = 25        # e-fragment length per block-diag group per plane
M = 5         # h-rows packed per column set (stationary free dim)
J = 2         # DoubleRow planes
KU = G * M    # 125 useful contraction rows per plane
K = 128       # padded contraction rows (zero rows 125-127)
NCH = E // (G * J)  # 6 e-chunks of 50
HG = H // M   # 60 h-groups
HB = 4        # h-groups per batch
NB = HG // HB  # 15 batches
NCOL = HB * P_LOC  # 512 output columns per matmul (= one PSUM bank f32)
STP = 16      # stationary plane stride (DoubleRow wants step % 16 == 0)

ALPHA = 256.0  # weight scale into e4m3 range
BETA = 32.0    # wi/wj scale into e4m3 range
INV_SCALE = 1.0 / (ALPHA * BETA)
F8_MAX = 240.0

# Set by test harness to capture a profile; kernel() stores results here.
TRACE = False
LAST_RESULT = None

_CACHE: dict = {}


def _build_nc():
    import concourse.bacc as bacc
    import concourse.mybir as mybir
    import concourse.tile as tile

    f32 = mybir.dt.float32
    f8 = mybir.dt.float8e4

    nc = bacc.Bacc("TRN2", debug=False)

    # Moving operands, host-pretransposed/quantized:
    #   row b*K+k, col c*(J*NCOL) + j*NCOL + hg_l*P_LOC + rel
    #     = Wq[rel, (b*HB+hg_l)*M + k//G, c*G*J + j*G + k%G]  (k < 125)
    lx = nc.dram_tensor("lx", [NB * K, NCH * J * NCOL], f8, kind="ExternalInput").ap()
    rx = nc.dram_tensor("rx", [NB * K, NCH * J * NCOL], f8, kind="ExternalInput").ap()
    # Block-diagonal stationaries: [128, (side*NCH+c)*2*STP + j*STP + m]
    st = nc.dram_tensor("st", [K, 2 * NCH * J * STP], f8, kind="ExternalInput").ap()
    rm = nc.dram_tensor("rm", [P_LOC, 2], f32, kind="ExternalInput").ap()
    idn = nc.dram_tensor("idn", [P_LOC, P_LOC], f32, kind="ExternalInput").ap()
    out = nc.dram_tensor("out", [P_LOC, 4], f32, kind="ExternalOutput").ap()

    with tile.TileContext(nc) as tc, ExitStack() as ctx:
        const_pool = ctx.enter_context(tc.tile_pool(name="const", bufs=1))
        lpool = ctx.enter_context(tc.tile_pool(name="lmov", bufs=8))
        rpool = ctx.enter_context(tc.tile_pool(name="rmov", bufs=8))
        spool = ctx.enter_context(tc.tile_pool(name="sq", bufs=3))
        psum_pool = ctx.enter_context(tc.tile_pool(name="ps", bufs=4, space="PSUM"))
        warm_pool = ctx.enter_context(tc.tile_pool(name="wps", bufs=1, space="PSUM"))
        gram_pool = ctx.enter_context(tc.tile_pool(name="gps", bufs=1, space="PSUM"))

        st_sb = const_pool.tile([K, 2 * NCH * J * STP], f8)
        nc.scalar.dma_start(st_sb[:], st[:])
        rm_sb = const_pool.tile([P_LOC, 2], f32)
        nc.scalar.dma_start(rm_sb[:], rm[:])
        idn_sb = const_pool.tile([P_LOC, P_LOC], f32)
        nc.scalar.dma_start(idn_sb[:], idn[:])

        # HAM pre-warm: dummy matmuls bridge the gap until the first batch
        # lands, so the PE clock-gate reaches 8/8 as real work starts. Kept
        # short: a long warm block would stall the PE FIFO while DMA fills
        # every pool slot and then idles on slot recycling.
        warm_ps = warm_pool.tile([M, 128], f32)
        for _ in range(14):
            nc.tensor.matmul(
                warm_ps[:], st_sb[:, 0:M], st_sb[:, 0:128], start=True, stop=True
            )

        gram = gram_pool.tile([P_LOC, P_LOC], f32)
        CW = J * NCOL  # 1024 columns per chunk
        for b in range(NB):
            last = b == NB - 1
            xt_l = lpool.tile([K, NCH * CW], f8, name="xt_l", tag="xt_l")
            # Both sides on the SP HWDGE ring: the ACT ring's issues would
            # queue behind ACT's evac semaphore waits. (Splitting the last
            # batch into piece-tiles was tried and measured slower: per-piece
            # issue overhead and small-transfer inefficiency at the stream
            # tail outweigh the finer matmul dependencies.)
            nc.sync.dma_start(xt_l[:], lx[b * K : (b + 1) * K, :])
            xt_r = rpool.tile([K, NCH * CW], f8, name="xt_r", tag="xt_r")
            nc.sync.dma_start(xt_r[:], rx[b * K : (b + 1) * K, :])
            r_src = lambda c: xt_r[:, c * CW : (c + 1) * CW]

            ps = psum_pool.tile([M, NCOL], f32, name="ps", tag="ps")
            n = 0
            for s in (0, 1):
                for c in range(NCH):
                    base = (s * NCH + c) * J * STP
                    lhsT = st_sb[:, base : base + J * STP].rearrange(
                        "p (j x) -> p j x", j=J
                    )[:, :, 0:M]
                    mov = xt_l[:, c * CW : (c + 1) * CW] if s == 0 else r_src(c)
                    nc.tensor.matmul(
                        ps[:],
                        lhsT,
                        mov.rearrange("p (j n) -> p j n", j=J),
                        start=(n == 0),
                        stop=(n == 2 * NCH - 1),
                        perf_mode=mybir.MatmulPerfMode.DoubleRow,
                    )
                    n += 1

            # Evacuate diff (scaled to true magnitude) as bf16; Gram matmuls
            # then accumulate sum_m diff^2 per rel on the diagonal of one
            # persistent PSUM tile: out[c,c'] += sum_m d[m,c]*d[m,c'].
            sq = spool.tile([M, NCOL], mybir.dt.bfloat16)
            nc.scalar.activation(
                sq[:], ps[:], mybir.ActivationFunctionType.Copy, scale=INV_SCALE
            )
            for g in range(HB):
                sl = sq[:, g * P_LOC : (g + 1) * P_LOC]
                nc.tensor.matmul(
                    gram[:],
                    sl,
                    sl,
                    start=(b == 0 and g == 0),
                    stop=(last and g == HB - 1),
                    skip_group_check=True,
                )

        # dist[rel] = diagonal of gram: one masked-reduce against identity.
        dist_sb = const_pool.tile([P_LOC, 1], f32)
        diag_tmp = const_pool.tile([P_LOC, P_LOC], f32)
        nc.vector.scalar_tensor_tensor(
            diag_tmp[:],
            gram[:],
            1.0,
            idn_sb[:],
            mybir.AluOpType.mult,
            mybir.AluOpType.mult,
            accum_out=dist_sb[:],
        )

        out_sb = const_pool.tile([P_LOC, 4], f32)
        nc.vector.tensor_scalar_mul(out_sb[:, 0:2], rm_sb[:, 0:2], dist_sb[:, 0:1])
        nc.vector.tensor_copy(out_sb[:, 2:4], rm_sb[:, 0:2])
        nc.sync.dma_start(out[:], out_sb[:])

    nc.compile()
    return nc


def _quant_f8(x):
    import ml_dtypes

    return np.clip(x, -F8_MAX, F8_MAX).astype(ml_dtypes.float8_e4m3)


def _prepack_side(wq_core):
    """[128, 300, 300] e4m3 -> [NB*K, NCH*J*NCOL] moving layout, K-padded."""
    # e = c*(G*J) + j*G + i
    a = wq_core.reshape(P_LOC, NB, HB, M, NCH, J, G)  # rel, b, hg_l, m, c, j, i
    a = a.transpose(1, 3, 6, 4, 5, 2, 0)  # b, m, i, c, j, hg_l, rel
    x = np.zeros((NB, K, NCH * J * NCOL), dtype=wq_core.dtype)
    x[:, :KU] = np.ascontiguousarray(a).reshape(NB, KU, NCH * J * NCOL)
    return x.reshape(NB * K, NCH * J * NCOL)


def kernel(tag_rep, Lp_w, Rp_w, relation, tag1_idx, tag2_idx):
    global LAST_RESULT
    from concourse.bass_utils import run_bass_kernel_spmd

    if "nc" not in _CACHE:
        _CACHE["nc"] = _build_nc()
    nc = _CACHE["nc"]

    tag_rep = np.asarray(tag_rep)
    rel = np.asarray(relation).astype(np.float32)  # values in {0, 1}

    wi = np.asarray(tag_rep[int(tag1_idx)], dtype=np.float32)
    wj = np.asarray(tag_rep[int(tag2_idx)], dtype=np.float32)
    v_l = _quant_f8(wi * BETA)
    v_r = _quant_f8(-wj * BETA)

    st = np.zeros((K, 2 * NCH * J * STP), dtype=v_l.dtype)
    for s, v in ((0, v_l), (1, v_r)):
        for c in range(NCH):
            for j in range(J):
                for m in range(M):
                    e0 = c * G * J + j * G
                    st[m * G : (m + 1) * G, (s * NCH + c) * J * STP + j * STP + m] = v[
                        e0 : e0 + G
                    ]

    lq = _quant_f8(np.asarray(Lp_w) * ALPHA)
    rq = _quant_f8(np.asarray(Rp_w) * ALPHA)

    in_maps = []
    for core in range(N_CORES):
        sl = slice(core * P_LOC, (core + 1) * P_LOC)
        rel_c = rel[sl]
        in_maps.append(
            {
                "lx": _prepack_side(lq[sl]),
                "rx": _prepack_side(rq[sl]),
                "st": st,
                "rm": np.ascontiguousarray(np.stack([rel_c, 1.0 - rel_c], axis=1)),
                "idn": np.eye(P_LOC, dtype=np.float32),
            }
        )

    kw = {}
    if TRACE:
        kw = dict(trace=True, trace_cores=[0])
    res = run_bass_kernel_spmd(nc, in_maps, core_ids=list(range(N_CORES)), **kw)
    LAST_RESULT = res

    out_full = np.empty((4, P_TOTAL), dtype=np.float32)
    for core in range(N_CORES):
        out_full[:, core * P_LOC : (core + 1) * P_LOC] = res.results[core]["out"].T
    return out_full


# revision 21
# speedup vs baseline: 1.1150x; 1.0010x over previous
"""Trainium2 Bass kernel for the KnowledgeGraphEmbedding loss.

Computes, for P=1024 relations sharded 128-per-core across 8 NeuronCores:
    li = Lp_w[p] @ wi          (wi = tag_rep[tag1_idx])
    rj = Rp_w[p] @ wj          (wj = tag_rep[tag2_idx])
    dist[p] = sum_h (li - rj)^2
    out = [dist*rel, dist*(1-rel), rel, 1-rel]   (rel in {0,1})

Memory-bound problem: all of Lp_w/Rp_w must stream from HBM once.
Strategy vs the f32 DVE baseline (~305us, at the f32 DMA roofline):

  1. Quantize weights host-side to fp8 e4m3 scaled by 256 into range;
     wi/-wj scaled by 32. Quarters HBM traffic to ~23.7 MB/core, which
     streams gap-free at 360-420 GB/s (the HBM/fabric ceiling; exact
     rate depends on phase alignment with the stack-paired core).
     End-to-end max-abs rel err 1.025e-2, deterministic (gate 2e-2).
  2. Multiply+reduce on the TensorEngine as a segmented matvec in
     DoubleRow mode (2 fp8 weights per PE cell = virtual 256-row array):
     stationary lhsT[128, 2, 5] is a block-diagonal of a 50-element
     wi/-wj e-chunk (5 groups of 2x25), moving rhs[128, 2, 512] carries
     the matching e-fragments of 5 h-rows x 512 (hg, rel) columns,
     host-pretransposed (rows 125-127 zero so the 128-partition DMA
     spray engages all 16 SDMA engines). 12 matmuls (2 sides x 6
     e-chunks) accumulate diff = li - rj in one PSUM bank per batch.
  3. ScalarE evacuates diff to bf16 SBUF (Copy, scale 1/8192 folds the
     quant scales); per h-group a K=5 Gram matmul (diff x diff) accumulates
     sum_m diff^2 on the diagonal of one persistent PSUM tile across all 60
     h-groups; one DVE scalar_tensor_tensor against an identity extracts
     the diagonal as dist[128, 1] with accum_out; mask/bin via DVE.
  Dummy matmuls at the start keep the PE HAM clock-gate warm while the
  first batches stream in.
"""

from contextlib import ExitStack

import numpy as np

N_CORES = 8
P_TOTAL = 1024
H = 300
E = 300
P_LOC = P_TOTAL // N_CORES  # 128 relations per core

G = 25        # e-fragment length per block-diag group per plane
M = 5         # h-rows packed per column set (stationary free dim)
J = 2         # DoubleRow planes
KU = G * M    # 125 useful contraction rows per plane
K = 128       # padded contraction rows (zero rows 125-127)
NCH = E // (G * J)  # 6 e-chunks of 50
HG = H // M   # 60 h-groups
HB = 4        # h-groups per batch
NB = HG // HB  # 15 batches
NCOL = HB * P_LOC  # 512 output columns per matmul (= one PSUM bank f32)
STP = 16      # stationary plane stride (DoubleRow wants step % 16 == 0)

ALPHA = 256.0  # weight scale into e4m3 range
BETA = 32.0    # wi/wj scale into e4m3 range
INV_SCALE = 1.0 / (ALPHA * BETA)
F8_MAX = 240.0

# Set by test harness to capture a profile; kernel() stores results here.
TRACE = False
LAST_RESULT = None

_CACHE: dict = {}


def _build_nc():
    import concourse.bacc as bacc
    import concourse.mybir as mybir
    import concourse.tile as tile

    f32 = mybir.dt.float32
    f8 = mybir.dt.float8e4

    nc = bacc.Bacc("TRN2", debug=False)

    # Moving operands, host-pretransposed/quantized:
    #   row b*K+k, col c*(J*NCOL) + j*NCOL + hg_l*P_LOC + rel
    #     = Wq[rel, (b*HB+hg_l)*M + k//G, c*G*J + j*G + k%G]  (k < 125)
    lx = nc.dram_tensor("lx", [NB * K, NCH * J * NCOL], f8, kind="ExternalInput").ap()
    rx = nc.dram_tensor("rx", [NB * K, NCH * J * NCOL], f8, kind="ExternalInput").ap()
    # Block-diagonal stationaries: [128, (side*NCH+c)*2*STP + j*STP + m]
    st = nc.dram_tensor("st", [K, 2 * NCH * J * STP], f8, kind="ExternalInput").ap()
    rm = nc.dram_tensor("rm", [P_LOC, 2], f32, kind="ExternalInput").ap()
    idn = nc.dram_tensor("idn", [P_LOC, P_LOC], f32, kind="ExternalInput").ap()
    out = nc.dram_tensor("out", [P_LOC, 4], f32, kind="ExternalOutput").ap()

    with tile.TileContext(nc) as tc, ExitStack() as ctx:
        const_pool = ctx.enter_context(tc.tile_pool(name="const", bufs=1))
        lpool = ctx.enter_context(tc.tile_pool(name="lmov", bufs=8))
        rpool = ctx.enter_context(tc.tile_pool(name="rmov", bufs=8))
        spool = ctx.enter_context(tc.tile_pool(name="sq", bufs=3))
        psum_pool = ctx.enter_context(tc.tile_pool(name="ps", bufs=4, space="PSUM"))
        warm_pool = ctx.enter_context(tc.tile_pool(name="wps", bufs=1, space="PSUM"))
        gram_pool = ctx.enter_context(tc.tile_pool(name="gps", bufs=1, space="PSUM"))

        st_sb = const_pool.tile([K, 2 * NCH * J * STP], f8)
        nc.scalar.dma_start(st_sb[:], st[:])
        rm_sb = const_pool.tile([P_LOC, 2], f32)
        nc.scalar.dma_start(rm_sb[:], rm[:])
        idn_sb = const_pool.tile([P_LOC, P_LOC], f32)
        nc.scalar.dma_start(idn_sb[:], idn[:])

        # HAM pre-warm: dummy matmuls bridge the gap until the first batch
        # lands, so the PE clock-gate reaches 8/8 as real work starts. Kept
        # short: a long warm block would stall the PE FIFO while DMA fills
        # every pool slot and then idles on slot recycling.
        warm_ps = warm_pool.tile([M, 128], f32)
        for _ in range(14):
            nc.tensor.matmul(
                warm_ps[:], st_sb[:, 0:M], st_sb[:, 0:128], start=True, stop=True
            )

        gram = gram_pool.tile([P_LOC, P_LOC], f32)
        CW = J * NCOL  # 1024 columns per chunk
        for b in range(NB):
            last = b == NB - 1
            xt_l = lpool.tile([K, NCH * CW], f8, name="xt_l", tag="xt_l")
            # Both sides on the SP HWDGE ring: the ACT ring's issues would
            # queue behind ACT's evac semaphore waits. (Splitting the last
            # batch into piece-tiles was tried and measured slower: per-piece
            # issue overhead and small-transfer inefficiency at the stream
            # tail outweigh the finer matmul dependencies.)
            nc.sync.dma_start(xt_l[:], lx[b * K : (b + 1) * K, :])
            xt_r = rpool.tile([K, NCH * CW], f8, name="xt_r", tag="xt_r")
            nc.sync.dma_start(xt_r[:], rx[b * K : (b + 1) * K, :])
            r_src = lambda c: xt_r[:, c * CW : (c + 1) * CW]

            ps = psum_pool.tile([M, NCOL], f32, name="ps", tag="ps")
            n = 0
            for s in (0, 1):
                for c in range(NCH):
                    base = (s * NCH + c) * J * STP
                    lhsT = st_sb[:, base : base + J * STP].rearrange(
                        "p (j x) -> p j x", j=J
                    )[:, :, 0:M]
                    mov = xt_l[:, c * CW : (c + 1) * CW] if s == 0 else r_src(c)
                    nc.tensor.matmul(
                        ps[:],
                        lhsT,
                        mov.rearrange("p (j n) -> p j n", j=J),
                        start=(n == 0),
                        stop=(n == 2 * NCH - 1),
                        perf_mode=mybir.MatmulPerfMode.DoubleRow,
                    )
                    n += 1

            # Evacuate diff (scaled to true magnitude) as bf16; Gram matmuls
            # then accumulate sum_m diff^2 per rel on the diagonal of one
            # persistent PSUM tile: out[c,c'] += sum_m d[m,c]*d[m,c'].
            sq = spool.tile([M, NCOL], mybir.dt.bfloat16)
            nc.scalar.activation(
                sq[:], ps[:], mybir.ActivationFunctionType.Copy, scale=INV_SCALE
            )
            for g in range(HB):
                sl = sq[:, g * P_LOC : (g + 1) * P_LOC]
                nc.tensor.matmul(
                    gram[:],
                    sl,
                    sl,
                    start=(b == 0 and g == 0),
                    stop=(last and g == HB - 1),
                    skip_group_check=True,
                )

        # dist[rel] = diagonal of gram: one masked-reduce against identity.
        dist_sb = const_pool.tile([P_LOC, 1], f32)
        diag_tmp = const_pool.tile([P_LOC, P_LOC], f32)
        nc.vector.scalar_tensor_tensor(
            diag_tmp[:],
            gram[:],
            1.0,
            idn_sb[:],
            mybir.AluOpType.mult,
            mybir.AluOpType.mult,
            accum_out=dist_sb[:],
        )

        out_sb = const_pool.tile([P_LOC, 4], f32)
        nc.vector.tensor_scalar_mul(out_sb[:, 0:2], rm_sb[:, 0:2], dist_sb[:, 0:1])
        nc.vector.tensor_copy(out_sb[:, 2:4], rm_sb[:, 0:2])
        nc.sync.dma_start(out[:], out_sb[:])

    nc.compile()
    return nc


def _quant_f8(x):
    import ml_dtypes

    return np.clip(x, -F8_MAX, F8_MAX).astype(ml_dtypes.float8_e4m3)


def _prepack_side(wq_core):
    """[128, 300, 300] e4m3 -> [NB*K, NCH*J*NCOL] moving layout, K-padded."""
    # e = c*(G*J) + j*G + i
    a = wq_core.reshape(P_LOC, NB, HB, M, NCH, J, G)  # rel, b, hg_l, m, c, j, i
    a = a.transpose(1, 3, 6, 4, 5, 2, 0)  # b, m, i, c, j, hg_l, rel
    x = np.zeros((NB, K, NCH * J * NCOL), dtype=wq_core.dtype)
    x[:, :KU] = np.ascontiguousarray(a).reshape(NB, KU, NCH * J * NCOL)
    return x.reshape(NB * K, NCH * J * NCOL)


def kernel(tag_rep, Lp_w, Rp_w, relation, tag1_idx, tag2_idx):
    global LAST_RESULT
    from concourse.bass_utils import run_bass_kernel_spmd

    if "nc" not in _CACHE:
        _CACHE["nc"] = _build_nc()
    nc = _CACHE["nc"]

    tag_rep = np.asarray(tag_rep)
    rel = np.asarray(relation).astype(np.float32)  # values in {0, 1}

    wi = np.asarray(tag_rep[int(tag1_idx)], dtype=np.float32)
    wj = np.asarray(tag_rep[int(tag2_idx)], dtype=np.float32)
    v_l = _quant_f8(wi * BETA)
    v_r = _quant_f8(-wj * BETA)

    st = np.zeros((K, 2 * NCH * J * STP), dtype=v_l.dtype)
    for s, v in ((0, v_l), (1, v_r)):
        for c in range(NCH):
            for j in range(J):
                for m in range(M):
                    e0 = c * G * J + j * G
                    st[m * G : (m + 1) * G, (s * NCH + c) * J * STP + j * STP + m] = v[
                        e0 : e0 + G
                    ]

    lq = _quant_f8(np.asarray(Lp_w) * ALPHA)
    rq = _quant_f8(np.asarray(Rp_w) * ALPHA)

    in_maps = []
    for core in range(N_CORES):
        sl = slice(core * P_LOC, (core + 1) * P_LOC)
        rel_c = rel[sl]
        in_maps.append(
            {
                "lx": _prepack_side(lq[sl]),
                "rx": _prepack_side(rq[sl]),
                "st": st,
                "rm": np.ascontiguousarray(np.stack([rel_c, 1.0 - rel_c], axis=1)),
                "idn": np.eye(P_LOC, dtype=np.float32),
            }
        )

    kw = {}
    if TRACE:
        kw = dict(trace=True, trace_cores=[0])
    res = run_bass_kernel_spmd(nc, in_maps, core_ids=list(range(N_CORES)), **kw)
    LAST_RESULT = res

    out_full = np.empty((4, P_TOTAL), dtype=np.float32)
    for core in range(N_CORES):
        out_full[:, core * P_LOC : (core + 1) * P_LOC] = res.results[core]["out"].T
    return out_full
